# revision 2
# baseline (speedup 1.0000x reference)
"""CCSDS-123 lossless compressor forward pass on 8 Trainium2 NeuronCores.

Sharding: spectral (Z) axis, 28 bands per core + 1 halo band below.

Key algebraic facts (all arithmetic is exact in fp32 here: every value is an
integer multiple of 1/8 with magnitude << 2^21):
  * In lossless mode the "causal" predictor is a pure local stencil of the
    original image: sigma = W + NW + N + NE (with CCSDS edge rules) and
    pred = 0.125*sigma + 0.5*prev_band (z>0), pred = 0.25*sigma (z==0).
  * reconstructed == sample_representatives == clip(image) == image exactly,
    and quantized_residuals == residuals exactly, so the device only needs to
    produce predictions, residuals and mapped_indices.
  * Feeding core 0 a halo band equal to the *spatial* prediction of band 0
    makes the uniform z>0 formula produce the correct band-0 output
    (0.125*(sigma + 4*(0.25*sigma)) == 0.25*sigma), so the SPMD program has
    no z==0 special case.

Device mapping per band (plane stored band-wide as [128, 4, 514] with a
zero column per 128-row chunk so the W(x-1) shift is a plain AP slice):
  * t = cur + cur_right, H = horizontal 3-tap with CCSDS edge rules folded
    into columns 0/511 (VectorE).
  * PSUM per chunk accumulates S1@H (vertical shift), the chunk-boundary /
    top-row term (E127@H_prev / E3@W), and 4*prev_band (I4) — TensorE
    matmuls with one-hot fp32 shift matrices; the fp32 PE path is exact.
  * s2 = psum + W (VectorE), pred = 0.125*s2 (ScalarE; origin pixel 0.25).
  * resid = cur - pred (VectorE); q = round-to-nearest-even via +-1.5*2^23
    on ScalarE; mapped = max(2q, -2q-1) on VectorE with int32 output cast.
"""

import os
import sys

for _p in ("/opt/trn_rl_repo", "/root/.axon_site/_ro/trn_rl_repo"):
    if os.path.isdir(_p) and _p not in sys.path:
        sys.path.insert(0, _p)

import numpy as np

import concourse.bacc as bacc
import concourse.mybir as mybir
from concourse import tile
from concourse.bass_utils import run_bass_kernel_spmd

F32 = mybir.dt.float32
I32 = mybir.dt.int32
COPY = mybir.ActivationFunctionType.Copy

Z, Y, X = 224, 512, 512
N_CORES = 8
BPC = Z // N_CORES          # bands per core
NCH = Y // 128              # 128-row chunks per band plane
XP = X + 2                  # per-chunk columns: [0, x0..x511, pad]
CRND = 12582912.0           # 1.5 * 2^23: fp32 round-to-nearest-even constant


def _build_weights() -> np.ndarray:
    """Stationary matrices, packed [128, 4*128] (lhsT: out = lhsT.T @ in).

    S1   : out[p] = in[p-1]   (vertical shift within a chunk)
    E127 : out[0] = in[127]   (chunk-boundary row)
    I4   : 4 * I              (previous-band term)
    E3   : out[0] = 3*in[0]   (top-row 4W rule)
    """
    S1 = np.zeros((128, 128), np.float32)
    for p in range(1, 128):
        S1[p - 1, p] = 1.0
    E127 = np.zeros((128, 128), np.float32)
    E127[127, 0] = 1.0
    I4 = 4.0 * np.eye(128, dtype=np.float32)
    E3 = np.zeros((128, 128), np.float32)
    E3[0, 0] = 3.0
    return np.concatenate([S1, E127, I4, E3], axis=1)


_WTS = _build_weights()


def _spatial_pred_band0(b: np.ndarray) -> np.ndarray:
    """Host fp32 spatial prediction of band 0 (exact; used as core 0's halo)."""
    b = b.astype(np.float32)
    W = np.zeros_like(b)
    W[:, 1:] = b[:, :-1]
    N = np.zeros_like(b)
    N[1:, :] = b[:-1, :]
    NW = np.zeros_like(b)
    NW[1:, 1:] = b[:-1, :-1]
    NE = np.zeros_like(b)
    NE[1:, :-1] = b[:-1, 1:]
    sigma = W + NW + N + NE
    sigma[0, 1:] = 4.0 * W[0, 1:]
    sigma[1:, 0] = 2.0 * (N[1:, 0] + NE[1:, 0])
    sigma[1:, -1] = W[1:, -1] + NW[1:, -1] + 2.0 * N[1:, -1]
    sigma[0, 0] = 0.0
    return (np.float32(0.25) * sigma).astype(np.float32)


_NC_CACHE = None


def _build_nc():
    nc = bacc.Bacc("TRN2")
    chunk_d = nc.dram_tensor("chunk", [BPC + 1, Y, X], F32, kind="ExternalInput")
    wts_d = nc.dram_tensor("wts", [128, 4 * 128], F32, kind="ExternalInput")
    pred_d = nc.dram_tensor("pred", [BPC, Y, X], F32, kind="ExternalOutput")
    resid_d = nc.dram_tensor("resid", [BPC, Y, X], F32, kind="ExternalOutput")
    mapped_d = nc.dram_tensor("mapped", [BPC, Y, X], I32, kind="ExternalOutput")

    with tile.TileContext(nc) as tc:
        with (
            tc.tile_pool(name="wpool", bufs=1) as wpool,
            tc.tile_pool(name="curp", bufs=4) as curp,
            tc.tile_pool(name="tmpp", bufs=2) as tmpp,
            tc.tile_pool(name="outp", bufs=2) as outp,
            tc.tile_pool(name="psp", bufs=8, space="PSUM") as psp,
        ):
            wts = wpool.tile([128, 4 * 128], F32)
            nc.sync.dma_start(wts[:], wts_d[:])
            W_S1 = wts[:, 0:128]
            W_E127 = wts[:, 128:256]
            W_I4 = wts[:, 256:384]
            W_E3 = wts[:, 384:512]

            cur_tiles = [None] * (BPC + 1)

            def load_band(z):
                c = curp.tile([128, NCH, XP], F32, tag="cur", name=f"cur{z}")
                nc.sync.dma_start(
                    c[:, :, 1 : X + 1],
                    chunk_d[z].rearrange("(c p) x -> p c x", p=128),
                )
                # zero W-column so the x-1 shift is a plain slice (pad col
                # is never read meaningfully; t/H edge fixes overwrite it)
                nc.vector.memset(c[:, :, 0:1], 0.0)
                cur_tiles[z] = c

            load_band(0)
            for z in range(1, BPC + 1):
                load_band(z)
                cur = cur_tiles[z]
                prev = cur_tiles[z - 1]

                t = tmpp.tile([128, NCH, X], F32, tag="ta", name=f"t{z}")
                H = tmpp.tile([128, NCH, X], F32, tag="tb", name=f"H{z}")
                # t[x] = cur[x] + cur[x+1]  (col 511 garbage, never used)
                nc.vector.tensor_add(t[:], cur[:, :, 1 : XP - 1], cur[:, :, 2:XP])
                # H[x] = cur[x-1] + cur[x] + cur[x+1]  (interior)
                nc.vector.tensor_add(
                    H[:, :, 1 : X - 1], t[:, :, 0 : X - 2], cur[:, :, 3 : X + 1]
                )
                # edge columns (CCSDS rules folded in):
                #   H[0] = 2*(cur[0]+cur[1])      -> left col sigma = 2*(N+NE)
                #   H[511] = cur[510] + 2*cur[511] -> right col sigma += extra N
                nc.vector.tensor_scalar_mul(H[:, :, 0:1], t[:, :, 0:1], 2.0)
                nc.vector.tensor_add(
                    H[:, :, X - 1 : X], t[:, :, X - 2 : X - 1], cur[:, :, X : X + 1]
                )

                s2 = tmpp.tile([128, NCH, X], F32, tag="tc", name=f"s2_{z}")
                for c in range(NCH):
                    ps = psp.tile([128, X], F32, tag="ps", name=f"ps{z}_{c}")
                    # vertical shift of the 3-tap row sums
                    nc.tensor.matmul(ps[:], W_S1, H[:, c], start=True, stop=False)
                    if c == 0:
                        # plane top row: sigma = 4W -> add 3W on row 0
                        nc.tensor.matmul(
                            ps[:], W_E3, cur[:, 0, 0:X], start=False, stop=False
                        )
                    else:
                        # boundary up-row from previous chunk's row 127
                        nc.tensor.matmul(
                            ps[:], W_E127, H[:, c - 1], start=False, stop=False
                        )
                    # previous band: + 4*prev
                    nc.tensor.matmul(
                        ps[:], W_I4, prev[:, c, 1 : X + 1], start=False, stop=True
                    )
                    # s2 = psum + W  (W = x-1 shift = zero-led slice)
                    nc.vector.tensor_add(s2[:, c], ps[:], cur[:, c, 0:X])

                pred = outp.tile([128, NCH, X], F32, tag="pred", name=f"pred{z}")
                nc.scalar.activation(pred[:], s2[:], COPY, scale=0.125)
                # origin pixel: pred = prev[0,0] = 0.25 * s2[0,0]
                nc.scalar.activation(
                    pred[0:1, 0, 0:1], s2[0:1, 0, 0:1], COPY, scale=0.25
                )

                resid = outp.tile([128, NCH, X], F32, tag="resid", name=f"res{z}")
                nc.vector.tensor_sub(resid[:], cur[:, :, 1 : X + 1], pred[:])

                r1 = tmpp.tile([128, NCH, X], F32, tag="ta", name=f"r1_{z}")
                q2 = tmpp.tile([128, NCH, X], F32, tag="tb", name=f"q2_{z}")
                m1 = tmpp.tile([128, NCH, X], F32, tag="tc", name=f"m1_{z}")
                # r1 = resid + 1.5*2^23 rounds to integer (RNE); q = r1 - C
                nc.scalar.activation(r1[:], resid[:], COPY, bias=CRND)
                # q2 = 2q, m1 = -2q-1; mapped = max(q2, m1) is the zigzag map
                nc.scalar.activation(q2[:], r1[:], COPY, scale=2.0, bias=-2.0 * CRND)
                nc.scalar.activation(
                    m1[:], r1[:], COPY, scale=-2.0, bias=2.0 * CRND - 1.0
                )
                mapped = outp.tile([128, NCH, X], I32, tag="mapped", name=f"map{z}")
                nc.vector.tensor_max(mapped[:], q2[:], m1[:])

                zo = z - 1
                nc.sync.dma_start(
                    pred_d[zo].rearrange("(c p) x -> p c x", p=128), pred[:]
                )
                nc.sync.dma_start(
                    resid_d[zo].rearrange("(c p) x -> p c x", p=128), resid[:]
                )
                nc.sync.dma_start(
                    mapped_d[zo].rearrange("(c p) x -> p c x", p=128), mapped[:]
                )

    nc.finalize()
    return nc


def _get_nc():
    global _NC_CACHE
    if _NC_CACHE is None:
        _NC_CACHE = _build_nc()
    return _NC_CACHE


def _make_in_maps(image: np.ndarray):
    in_maps = []
    for m in range(N_CORES):
        chunk = np.empty((BPC + 1, Y, X), np.float32)
        chunk[0] = (
            _spatial_pred_band0(image[0]) if m == 0 else image[m * BPC - 1]
        )
        chunk[1:] = image[m * BPC : (m + 1) * BPC]
        in_maps.append({"chunk": chunk, "wts": _WTS})
    return in_maps


def kernel(image: np.ndarray):
    image = np.ascontiguousarray(image, dtype=np.float32)
    assert image.shape == (Z, Y, X), image.shape

    nc = _get_nc()
    in_maps = _make_in_maps(image)
    res = run_bass_kernel_spmd(nc, in_maps, core_ids=list(range(N_CORES)))

    predictions = np.concatenate([r["pred"] for r in res.results], axis=0)
    residuals = np.concatenate([r["resid"] for r in res.results], axis=0)
    mapped = np.concatenate([r["mapped"] for r in res.results], axis=0)
    reconstructed = np.clip(image, -32768.0, 32767.0).astype(np.float32)
    # lossless mode identities: quantized == residuals, sample reps == recon
    return (predictions, residuals, residuals, mapped, reconstructed, reconstructed)
